# revision 1
# baseline (speedup 1.0000x reference)
# Cross-attention kernel for Trainium2 (Bass/Tile), 8-core data-parallel.
#
# Reference computation (per batch element, B=8 -> one batch element per core):
#   q = x1 @ Wq.T + bq ; k = x2 @ Wk.T + bk ; v = x3 @ Wv.T + bv
#   out = softmax(q @ k.T) @ v          (no 1/sqrt(d) scale)
#
# Precision strategy (validated numerically against the fp32 reference,
# absmax rel err ~4e-3):
#   - q,k projections and q@k.T run as 3-pass bf16 hi/lo split matmuls
#     (hi = bf16(x), lo = bf16(x - hi); x@y ~= xh@yh + xh@yl + xl@yh),
#     accumulated in fp32 PSUM. Effective precision ~fp32 for the scores,
#     which matters because the unscaled scores have std ~46 and the softmax
#     is extremely sharp.
#   - v projection and attn@v run in plain bf16 (error contribution ~2e-3).
#   - softmax itself is fp32 (row max subtraction on-chip, exp on ScalarE,
#     normalization deferred to the output).
#
# Layout strategy per core (S=2048, C=1024, P=128):
#   - qT, kT computed directly transposed ([d, s], d on partitions) so the
#     score matmul contracts over d. All transposes (W, x, p) run on the DMA
#     xbar (2-byte transpose mode, SP HWDGE queue) -- they never touch the PE
#     or vector engines.
#   - v computed in natural [s, c] layout (stationary operand for attn@v).
#   - kT(hi/lo) and v stay resident in SBUF; qT(hi/lo) spills to a DRAM
#     scratch and streams back per 128-row query tile (ACT HWDGE queue, so
#     plain copies and xbar transposes live on different queues).
#   - p = exp(s - rowmax) transposed per sq-tile in one xbar DMA; the row sum
#     rides along via the activation accumulator, output normalized at the end.

from contextlib import ExitStack

import numpy as np

import concourse.bass as bass
import concourse.mybir as mybir
import concourse.tile as tile
from concourse import bacc
from concourse.bass_utils import run_bass_kernel_spmd

F32 = mybir.dt.float32
BF16 = mybir.dt.bfloat16
ADD = mybir.AluOpType.add
SUB = mybir.AluOpType.subtract
AX = mybir.AxisListType.X
EXP = mybir.ActivationFunctionType.Exp

B, S, C = 8, 2048, 1024
P = 128
NT_S = S // P  # 16 s-tiles
NT_C = C // P  # 8 c/d-tiles
CH = 512  # free-dim chunk (one fp32 PSUM bank)
NCH_S = S // CH  # 4
NCH_C = C // CH  # 2


def _emit(tc):
    nc = tc.nc

    x1 = nc.dram_tensor("x1", [S, C], F32, kind="ExternalInput").ap()
    x2 = nc.dram_tensor("x2", [S, C], F32, kind="ExternalInput").ap()
    x3 = nc.dram_tensor("x3", [S, C], F32, kind="ExternalInput").ap()
    Wq = nc.dram_tensor("Wq", [C, C], F32, kind="ExternalInput").ap()
    Wk = nc.dram_tensor("Wk", [C, C], F32, kind="ExternalInput").ap()
    Wv = nc.dram_tensor("Wv", [C, C], F32, kind="ExternalInput").ap()
    bq = nc.dram_tensor("bq", [C], F32, kind="ExternalInput").ap()
    bk = nc.dram_tensor("bk", [C], F32, kind="ExternalInput").ap()
    bv = nc.dram_tensor("bv", [C], F32, kind="ExternalInput").ap()
    out = nc.dram_tensor("out", [S, C], F32, kind="ExternalOutput").ap()

    es = ExitStack()
    with es:
        const = es.enter_context(tc.tile_pool(name="const", bufs=1))
        dram = es.enter_context(tc.tile_pool(name="dram", bufs=1, space="DRAM"))

        # biases: bq/bk as per-d-tile columns [128, 8]; bv broadcast [128, C]
        bq_sb = const.tile([P, NT_C], F32, tag="bq")
        nc.scalar.dma_start(out=bq_sb, in_=bq.rearrange("(t p) -> p t", p=P))
        bk_sb = const.tile([P, NT_C], F32, tag="bk")
        nc.scalar.dma_start(out=bk_sb, in_=bk.rearrange("(t p) -> p t", p=P))
        bv_sb = const.tile([P, C], F32, tag="bv")
        bv_bcast = bass.AP(tensor=bv.tensor, offset=bv.offset, ap=[[0, P], [1, C]])
        nc.scalar.dma_start(out=bv_sb, in_=bv_bcast)

        # DRAM scratch for spilled qT (hi/lo)
        qTh_d = dram.tile([NT_C, P, S], BF16, tag="qThd", name="qThd")
        qTl_d = dram.tile([NT_C, P, S], BF16, tag="qTld", name="qTld")

        def prep_w(W, wpool, split):
            """Load W [C,C] (rows d, cols c); produce W^T as one 3D tile
            [128c, NT_C(ct), C(d)] bf16 hi (and lo) via xbar transposes."""
            WhT = wpool.tile([P, NT_C, C], BF16, tag="WhT", name="WhT")
            WlT = wpool.tile([P, NT_C, C], BF16, tag="WlT", name="WlT") if split else None
            with tc.tile_pool(name="wstage", bufs=2) as ws:
                for dt in range(NT_C):
                    wnat = ws.tile([P, C], F32, tag="wnat", name="wnat")
                    weng = nc.gpsimd if dt % 2 == 0 else nc.scalar
                    weng.dma_start(out=wnat, in_=W[dt * P : (dt + 1) * P, :])
                    wh = ws.tile([P, C], BF16, tag="wh", name="wh")
                    nc.vector.tensor_copy(out=wh, in_=wnat)
                    nc.sync.dma_start(
                        out=WhT[:, :, dt * P : (dt + 1) * P], in_=wh, transpose=True
                    )
                    if split:
                        wl = ws.tile([P, C], BF16, tag="wl", name="wl")
                        nc.vector.tensor_tensor(out=wl, in0=wnat, in1=wh, op=SUB)
                        nc.sync.dma_start(
                            out=WlT[:, :, dt * P : (dt + 1) * P], in_=wl, transpose=True
                        )
            return WhT, WlT

        def prep_xT_chunk(x, s0, split, xs_pool, xt_pool):
            """Load x[s0:s0+CH, :] one s-tile at a time, split hi/lo, and xbar-
            transpose into [128c, NT_C(ct), CH(s)] bf16 tiles (hi, lo)."""
            nj = CH // P  # 4 s-tiles per chunk
            xhT = xt_pool.tile([P, NT_C, CH], BF16, tag="xhT", name="xhT")
            xlT = (
                xt_pool.tile([P, NT_C, CH], BF16, tag="xlT", name="xlT")
                if split
                else None
            )
            for j in range(nj):
                r0 = s0 + j * P
                xs = xs_pool.tile([P, C], F32, tag="xload", name="xload")
                xeng = nc.gpsimd if j % 2 == 0 else nc.scalar
                xeng.dma_start(out=xs, in_=x[r0 : r0 + P, :])
                xh = xs_pool.tile([P, C], BF16, tag="xh", name="xh")
                nc.vector.tensor_copy(out=xh, in_=xs)
                nc.sync.dma_start(
                    out=xhT[:, :, j * P : (j + 1) * P], in_=xh, transpose=True
                )
                if split:
                    xl = xs_pool.tile([P, C], BF16, tag="xl", name="xl")
                    nc.vector.tensor_tensor(out=xl, in0=xs, in1=xh, op=SUB)
                    nc.sync.dma_start(
                        out=xlT[:, :, j * P : (j + 1) * P], in_=xl, transpose=True
                    )
            return xhT, xlT

        def split_proj_mms(ps, xhT, xlT, WhT, WlT, dt):
            """Emit the 24 matmuls of a 3-pass split projection into psum ps."""
            n_mm = NT_C * 3
            i = 0
            for ct in range(NT_C):
                lw_h = WhT[:, ct, dt * P : (dt + 1) * P]
                nc.tensor.matmul(
                    ps, lw_h, xhT[:, ct, :], start=(i == 0), stop=(i == n_mm - 1)
                )
                i += 1
                nc.tensor.matmul(
                    ps, lw_h, xlT[:, ct, :], start=False, stop=(i == n_mm - 1)
                )
                i += 1
                lw_l = WlT[:, ct, dt * P : (dt + 1) * P]
                nc.tensor.matmul(
                    ps, lw_l, xhT[:, ct, :], start=False, stop=(i == n_mm - 1)
                )
                i += 1

        # ---------------- Phase Q: project qT (hi/lo) -> DRAM scratch --------
        with tc.tile_pool(name="wq", bufs=1) as wq_pool:
            WqhT, WqlT = prep_w(Wq, wq_pool, split=True)
            with (
                tc.tile_pool(name="qxs", bufs=2) as qxs,
                tc.tile_pool(name="qxt", bufs=2) as qxt,
                tc.tile_pool(name="qmmps", bufs=2, space="PSUM") as qmmps,
                tc.tile_pool(name="qst", bufs=3) as qst,
            ):
                nxt = prep_xT_chunk(x1, 0, True, qxs, qxt)
                for ich in range(NCH_S):
                    s0 = ich * CH
                    xhT, xlT = nxt
                    if ich + 1 < NCH_S:
                        nxt = prep_xT_chunk(x1, (ich + 1) * CH, True, qxs, qxt)
                    for dt in range(NT_C):
                        ps = qmmps.tile([P, CH], F32, tag="projps", name="projps")
                        split_proj_mms(ps, xhT, xlT, WqhT, WqlT, dt)
                        t = qst.tile([P, CH], F32, tag="projt", name="projt")
                        nc.vector.tensor_scalar_add(
                            out=t, in0=ps, scalar1=bq_sb[:, dt : dt + 1]
                        )
                        h = qst.tile([P, CH], BF16, tag="projh", name="projh")
                        nc.scalar.copy(out=h, in_=t)
                        l = qst.tile([P, CH], BF16, tag="projl", name="projl")
                        nc.vector.tensor_tensor(out=l, in0=t, in1=h, op=SUB)
                        nc.scalar.dma_start(out=qTh_d[dt, :, s0 : s0 + CH], in_=h)
                        nc.scalar.dma_start(out=qTl_d[dt, :, s0 : s0 + CH], in_=l)

        # ---------------- Phase K: project kT (hi/lo) -> resident SBUF -------
        res_k = es.enter_context(tc.tile_pool(name="resk", bufs=1))
        kTh = [
            res_k.tile([P, S], BF16, tag=f"kTh{i}", name=f"kTh{i}")
            for i in range(NT_C)
        ]
        kTl = [
            res_k.tile([P, S], BF16, tag=f"kTl{i}", name=f"kTl{i}")
            for i in range(NT_C)
        ]
        with tc.tile_pool(name="wk", bufs=1) as wk_pool:
            WkhT, WklT = prep_w(Wk, wk_pool, split=True)
            with (
                tc.tile_pool(name="kxs", bufs=2) as kxs,
                tc.tile_pool(name="kxt", bufs=2) as kxt,
                tc.tile_pool(name="kmmps", bufs=2, space="PSUM") as kmmps,
                tc.tile_pool(name="kst", bufs=3) as kst,
            ):
                nxt = prep_xT_chunk(x2, 0, True, kxs, kxt)
                for ich in range(NCH_S):
                    s0 = ich * CH
                    xhT, xlT = nxt
                    if ich + 1 < NCH_S:
                        nxt = prep_xT_chunk(x2, (ich + 1) * CH, True, kxs, kxt)
                    for dt in range(NT_C):
                        ps = kmmps.tile([P, CH], F32, tag="projps", name="kprojps")
                        split_proj_mms(ps, xhT, xlT, WkhT, WklT, dt)
                        t = kst.tile([P, CH], F32, tag="projt", name="kprojt")
                        nc.vector.tensor_scalar_add(
                            out=t, in0=ps, scalar1=bk_sb[:, dt : dt + 1]
                        )
                        h_sl = kTh[dt][:, s0 : s0 + CH]
                        nc.scalar.copy(out=h_sl, in_=t)
                        nc.vector.tensor_tensor(
                            out=kTl[dt][:, s0 : s0 + CH], in0=t, in1=h_sl, op=SUB
                        )

        # ---------------- Phase V: project v (natural [s, c]) -> resident ----
        res_v = es.enter_context(tc.tile_pool(name="resv", bufs=1))
        v_res = [
            res_v.tile([P, C], BF16, tag=f"v{i}", name=f"v{i}") for i in range(NT_S)
        ]
        with tc.tile_pool(name="wv", bufs=1) as wv_pool:
            WvhT, _ = prep_w(Wv, wv_pool, split=False)
            with (
                tc.tile_pool(name="vxs", bufs=2) as vxs,
                tc.tile_pool(name="vxt", bufs=2) as vxt,
                tc.tile_pool(name="vmmps", bufs=2, space="PSUM") as vmmps,
            ):
                nxt3 = prep_xT_chunk(x3, 0, False, vxs, vxt)
                for ich in range(NCH_S):
                    s0 = ich * CH
                    x3hT, _ = nxt3
                    if ich + 1 < NCH_S:
                        nxt3 = prep_xT_chunk(x3, (ich + 1) * CH, False, vxs, vxt)
                    for j in range(CH // P):  # s-tile within chunk
                        st = ich * (CH // P) + j
                        for cch in range(NCH_C):
                            ps = vmmps.tile([P, CH], F32, tag="vps", name="vps")
                            for ct in range(NT_C):
                                nc.tensor.matmul(
                                    ps,
                                    x3hT[:, ct, j * P : (j + 1) * P],
                                    WvhT[:, ct, cch * CH : (cch + 1) * CH],
                                    start=(ct == 0),
                                    stop=(ct == NT_C - 1),
                                )
                            nc.vector.tensor_tensor(
                                out=v_res[st][:, cch * CH : (cch + 1) * CH],
                                in0=ps,
                                in1=bv_sb[:, cch * CH : (cch + 1) * CH],
                                op=ADD,
                            )

        # ---------------- Attention ------------------------------------------
        with (
            tc.tile_pool(name="qstream", bufs=2) as qstream,
            tc.tile_pool(name="spsum", bufs=6, space="PSUM") as spsum,
            tc.tile_pool(name="opsum", bufs=2, space="PSUM") as opsum,
            tc.tile_pool(name="attn", bufs=2) as attn,
            tc.tile_pool(name="stats", bufs=4) as stats,
        ):
            for sq in range(NT_S):
                qh_t = qstream.tile([P, NT_C, P], BF16, tag="qh", name="qh")
                nc.scalar.dma_start(
                    out=qh_t,
                    in_=qTh_d[:, :, sq * P : (sq + 1) * P].rearrange("t p s -> p t s"),
                )
                ql_t = qstream.tile([P, NT_C, P], BF16, tag="ql", name="ql")
                nc.scalar.dma_start(
                    out=ql_t,
                    in_=qTl_d[:, :, sq * P : (sq + 1) * P].rearrange("t p s -> p t s"),
                )

                # scores: s[sq-tile, :] accumulated over d in 4 chunk banks
                ps_s = [
                    spsum.tile([P, CH], F32, tag="s", name=f"s{c}")
                    for c in range(NCH_S)
                ]
                cnt = [0] * NCH_S
                n_per = NT_C * 3
                for dt in range(NT_C):
                    qh_sl = qh_t[:, dt, :]
                    ql_sl = ql_t[:, dt, :]
                    for c in range(NCH_S):
                        nc.tensor.matmul(
                            ps_s[c],
                            qh_sl,
                            kTh[dt][:, c * CH : (c + 1) * CH],
                            start=(cnt[c] == 0),
                            stop=(cnt[c] == n_per - 1),
                        )
                        cnt[c] += 1
                    for c in range(NCH_S):
                        nc.tensor.matmul(
                            ps_s[c],
                            qh_sl,
                            kTl[dt][:, c * CH : (c + 1) * CH],
                            start=False,
                            stop=(cnt[c] == n_per - 1),
                        )
                        cnt[c] += 1
                    for c in range(NCH_S):
                        nc.tensor.matmul(
                            ps_s[c],
                            ql_sl,
                            kTh[dt][:, c * CH : (c + 1) * CH],
                            start=False,
                            stop=(cnt[c] == n_per - 1),
                        )
                        cnt[c] += 1

                # softmax (fp32, row-wise over the free dim)
                mx = stats.tile([P, NCH_S], F32, tag="mx", name="mx")
                for c in range(NCH_S):
                    nc.vector.reduce_max(out=mx[:, c : c + 1], in_=ps_s[c], axis=AX)
                negmax = stats.tile([P, 1], F32, tag="negmax", name="negmax")
                nc.vector.reduce_max(out=negmax, in_=mx, axis=AX, negate=True)

                p_sb = attn.tile([P, S], BF16, tag="p", name="p")
                sums = stats.tile([P, NCH_S], F32, tag="sums", name="sums")
                for c in range(NCH_S):
                    nc.scalar.activation(
                        out=p_sb[:, c * CH : (c + 1) * CH],
                        in_=ps_s[c],
                        func=EXP,
                        bias=negmax,
                        scale=1.0,
                        accum_out=sums[:, c : c + 1],
                    )
                rs = stats.tile([P, 1], F32, tag="rs", name="rs")
                nc.vector.reduce_sum(out=rs, in_=sums, axis=AX)
                rinv = stats.tile([P, 1], F32, tag="rinv", name="rinv")
                nc.vector.reciprocal(out=rinv, in_=rs)

                # transpose p for attn @ v: one xbar DMA per sq-tile
                pT = attn.tile([P, NT_S, P], BF16, tag="pT", name="pT")
                nc.sync.dma_start(out=pT, in_=p_sb, transpose=True)

                # attn @ v, accumulate over sk tiles; normalize; store
                ps_o = [
                    opsum.tile([P, CH], F32, tag="o", name=f"o{c}")
                    for c in range(NCH_C)
                ]
                for skt in range(NT_S):
                    for cch in range(NCH_C):
                        nc.tensor.matmul(
                            ps_o[cch],
                            pT[:, skt, :],
                            v_res[skt][:, cch * CH : (cch + 1) * CH],
                            start=(skt == 0),
                            stop=(skt == NT_S - 1),
                        )
                o_sb = attn.tile([P, C], F32, tag="osb", name="osb")
                for cch in range(NCH_C):
                    nc.vector.tensor_scalar_mul(
                        out=o_sb[:, cch * CH : (cch + 1) * CH],
                        in0=ps_o[cch],
                        scalar1=rinv,
                    )
                nc.scalar.dma_start(out=out[sq * P : (sq + 1) * P, :], in_=o_sb)


_BUILT = {}


def _build():
    if "nc" not in _BUILT:
        nc = bacc.Bacc(
            "TRN2",
            target_bir_lowering=False,
            debug=False,
            num_devices=B,
        )
        with tile.TileContext(nc) as tc:
            _emit(tc)
        nc.compile()
        _BUILT["nc"] = nc
    return _BUILT["nc"]


def kernel_with_results(trace=False, **inputs):
    nc = _build()
    in_maps = []
    for i in range(B):
        in_maps.append(
            {
                "x1": np.ascontiguousarray(inputs["x1"][i], dtype=np.float32),
                "x2": np.ascontiguousarray(inputs["x2"][i], dtype=np.float32),
                "x3": np.ascontiguousarray(inputs["x3"][i], dtype=np.float32),
                "Wq": np.ascontiguousarray(inputs["Wq"], dtype=np.float32),
                "Wk": np.ascontiguousarray(inputs["Wk"], dtype=np.float32),
                "Wv": np.ascontiguousarray(inputs["Wv"], dtype=np.float32),
                "bq": np.ascontiguousarray(inputs["bq"], dtype=np.float32),
                "bk": np.ascontiguousarray(inputs["bk"], dtype=np.float32),
                "bv": np.ascontiguousarray(inputs["bv"], dtype=np.float32),
            }
        )
    res = run_bass_kernel_spmd(nc, in_maps, core_ids=list(range(B)), trace=trace)
    outs = np.stack([r["out"] for r in res.results], axis=0).astype(np.float32)
    return outs, res


def kernel(**inputs):
    outs, _ = kernel_with_results(trace=False, **inputs)
    return outs



# revision 2
# speedup vs baseline: 1.3559x; 1.3559x over previous
# Cross-attention kernel for Trainium2 (Bass/Tile), 8-core data-parallel.
#
# Reference computation (per batch element, B=8 -> one batch element per core):
#   q = x1 @ Wq.T + bq ; k = x2 @ Wk.T + bk ; v = x3 @ Wv.T + bv
#   out = softmax(q @ k.T) @ v          (no 1/sqrt(d) scale)
#
# Precision strategy (validated numerically against the fp32 reference and on
# hardware, absmax rel err ~1.1e-2 vs the 2e-2 gate):
#   - q,k projections and q@k.T run as SINGLE-pass fp32r matmuls. fp32r is
#     fp32 rounded to 11 explicit mantissa bits (measured on HW); the PE runs
#     it at full bf16 rate when the moving free dim is >= 256, and the matmul
#     is exact given the rounded inputs. This replaces the previous 3-pass
#     bf16 hi/lo splits (15 matmul passes -> 7).
#   - v projection and attn@v run in fp16 (full rate, ~3e-4 contribution).
#   - softmax is fp32 (row max on DVE, exp on ScalarE with accumulate,
#     normalization deferred to the output).
#
# Layout strategy per core (S=2048, C=1024, P=128):
#   - qT, kT computed directly transposed ([d, s], d on partitions) so the
#     score matmul contracts over d. x and W reach the transposed layout via
#     bf16 hi/lo split + 2-byte DMA xbar transposes, recombined to fp32r by a
#     single DVE add (fp32r == bf16hi + bf16lo exactly). x3/Wv go through a
#     single fp16 convert + xbar transpose.
#   - kT (fp32r) and v (fp16) stay resident in SBUF; qT (fp32r) spills to a
#     DRAM scratch and streams back per 128-row query tile.
#   - p = exp(s - rowmax) stored fp16, transposed per sq-tile in one xbar DMA;
#     row sums ride the activation accumulator; output normalized at the end.

from contextlib import ExitStack

import numpy as np

import concourse.bass as bass
import concourse.mybir as mybir
import concourse.tile as tile
from concourse import bacc
from concourse.bass_utils import run_bass_kernel_spmd

F32 = mybir.dt.float32
F32R = mybir.dt.float32r
BF16 = mybir.dt.bfloat16
F16 = mybir.dt.float16
ADD = mybir.AluOpType.add
SUB = mybir.AluOpType.subtract
AX = mybir.AxisListType.X
EXP = mybir.ActivationFunctionType.Exp

B, S, C = 8, 2048, 1024
P = 128
NT_S = S // P  # 16 s-tiles
NT_C = C // P  # 8 c/d-tiles
CH = 512  # free-dim chunk (one fp32 PSUM bank; fp32r full rate needs >=256)
NCH_S = S // CH  # 4
NCH_C = C // CH  # 2


def _emit(tc):
    nc = tc.nc

    x1 = nc.dram_tensor("x1", [S, C], F32, kind="ExternalInput").ap()
    x2 = nc.dram_tensor("x2", [S, C], F32, kind="ExternalInput").ap()
    x3 = nc.dram_tensor("x3", [S, C], F32, kind="ExternalInput").ap()
    Wq = nc.dram_tensor("Wq", [C, C], F32, kind="ExternalInput").ap()
    Wk = nc.dram_tensor("Wk", [C, C], F32, kind="ExternalInput").ap()
    Wv = nc.dram_tensor("Wv", [C, C], F32, kind="ExternalInput").ap()
    bq = nc.dram_tensor("bq", [C], F32, kind="ExternalInput").ap()
    bk = nc.dram_tensor("bk", [C], F32, kind="ExternalInput").ap()
    bv = nc.dram_tensor("bv", [C], F32, kind="ExternalInput").ap()
    out = nc.dram_tensor("out", [S, C], F32, kind="ExternalOutput").ap()

    es = ExitStack()
    with es:
        const = es.enter_context(tc.tile_pool(name="const", bufs=1))
        dram = es.enter_context(tc.tile_pool(name="dram", bufs=1, space="DRAM"))

        # biases: bq/bk as per-d-tile columns [128, 8]; bv broadcast [128, C]
        bq_sb = const.tile([P, NT_C], F32, tag="bq")
        nc.scalar.dma_start(out=bq_sb, in_=bq.rearrange("(t p) -> p t", p=P))
        bk_sb = const.tile([P, NT_C], F32, tag="bk")
        nc.scalar.dma_start(out=bk_sb, in_=bk.rearrange("(t p) -> p t", p=P))
        bv_sb = const.tile([P, C], F32, tag="bv")
        bv_bcast = bass.AP(tensor=bv.tensor, offset=bv.offset, ap=[[0, P], [1, C]])
        nc.scalar.dma_start(out=bv_sb, in_=bv_bcast)

        # DRAM scratch for spilled qT (fp32r bits)
        qT_d = dram.tile([NT_C, P, S], F32R, tag="qTd", name="qTd")

        def prep_w(W, wpool, f32r):
            """Load W [C,C] (rows d, cols c); produce W^T as one 3D tile
            [128c, NT_C(ct), C(d)] in fp32r (via bf16 hi/lo xbar transposes +
            recombine) or fp16 (single convert + transpose)."""
            WT = wpool.tile([P, NT_C, C], F32R if f32r else F16, tag="WT", name="WT")
            with tc.tile_pool(name="wstage", bufs=2) as ws:
                for dt in range(NT_C):
                    wnat = ws.tile([P, C], F32, tag="wnat", name="wnat")
                    weng = nc.gpsimd if dt % 2 == 0 else nc.scalar
                    weng.dma_start(out=wnat, in_=W[dt * P : (dt + 1) * P, :])
                    if f32r:
                        wh = ws.tile([P, C], BF16, tag="wh", name="wh")
                        nc.scalar.copy(out=wh, in_=wnat)
                        wl = ws.tile([P, C], BF16, tag="wl", name="wl")
                        nc.vector.tensor_tensor(out=wl, in0=wnat, in1=wh, op=SUB)
                        whT = ws.tile([P, NT_C, P], BF16, tag="whT", name="whT")
                        nc.sync.dma_start(out=whT, in_=wh, transpose=True)
                        wlT = ws.tile([P, NT_C, P], BF16, tag="wlT", name="wlT")
                        nc.sync.dma_start(out=wlT, in_=wl, transpose=True)
                        nc.vector.tensor_tensor(
                            out=WT[:, :, dt * P : (dt + 1) * P],
                            in0=whT,
                            in1=wlT,
                            op=ADD,
                        )
                    else:
                        wh = ws.tile([P, C], F16, tag="wh16", name="wh16")
                        nc.vector.tensor_copy(out=wh, in_=wnat)
                        nc.sync.dma_start(
                            out=WT[:, :, dt * P : (dt + 1) * P], in_=wh, transpose=True
                        )
            return WT

        def prep_xT_f32r(x, s0, xs_pool, xt_pool):
            """Load x[s0:s0+CH, :] one s-tile at a time; bf16 hi/lo split,
            xbar transpose both, recombine to one [128c, NT_C(ct), CH(s)]
            fp32r tile."""
            xT = xt_pool.tile([P, NT_C, CH], F32R, tag="xT", name="xT")
            for j in range(CH // P):
                r0 = s0 + j * P
                xs = xs_pool.tile([P, C], F32, tag="xload", name="xload")
                xeng = nc.gpsimd if j % 2 == 0 else nc.scalar
                xeng.dma_start(out=xs, in_=x[r0 : r0 + P, :])
                xh = xs_pool.tile([P, C], BF16, tag="xh", name="xh")
                nc.scalar.copy(out=xh, in_=xs)
                xl = xs_pool.tile([P, C], BF16, tag="xl", name="xl")
                nc.vector.tensor_tensor(out=xl, in0=xs, in1=xh, op=SUB)
                xhT = xs_pool.tile([P, NT_C, P], BF16, tag="xhT", name="xhT")
                nc.sync.dma_start(out=xhT, in_=xh, transpose=True)
                xlT = xs_pool.tile([P, NT_C, P], BF16, tag="xlT", name="xlT")
                nc.sync.dma_start(out=xlT, in_=xl, transpose=True)
                nc.vector.tensor_tensor(
                    out=xT[:, :, j * P : (j + 1) * P], in0=xhT, in1=xlT, op=ADD
                )
            return xT

        def prep_xT_f16(x, s0, xs_pool, xt_pool):
            """Load x[s0:s0+CH, :]; single fp16 convert + xbar transpose into
            [128c, NT_C(ct), CH(s)] fp16."""
            xT = xt_pool.tile([P, NT_C, CH], F16, tag="xT16", name="xT16")
            for j in range(CH // P):
                r0 = s0 + j * P
                xs = xs_pool.tile([P, C], F32, tag="xload3", name="xload3")
                xeng = nc.gpsimd if j % 2 == 0 else nc.scalar
                xeng.dma_start(out=xs, in_=x[r0 : r0 + P, :])
                xh = xs_pool.tile([P, C], F16, tag="xh3", name="xh3")
                nc.vector.tensor_copy(out=xh, in_=xs)
                nc.sync.dma_start(
                    out=xT[:, :, j * P : (j + 1) * P], in_=xh, transpose=True
                )
            return xT

        # ---------------- Phase Q: project qT (fp32r) -> DRAM scratch --------
        with tc.tile_pool(name="wq", bufs=1) as wq_pool:
            WqT = prep_w(Wq, wq_pool, f32r=True)
            with (
                tc.tile_pool(name="qxs", bufs=2) as qxs,
                tc.tile_pool(name="qxt", bufs=2) as qxt,
                tc.tile_pool(name="qmmps", bufs=2, space="PSUM") as qmmps,
                tc.tile_pool(name="qst", bufs=3) as qst,
            ):
                nxt = prep_xT_f32r(x1, 0, qxs, qxt)
                for ich in range(NCH_S):
                    s0 = ich * CH
                    xT = nxt
                    if ich + 1 < NCH_S:
                        nxt = prep_xT_f32r(x1, (ich + 1) * CH, qxs, qxt)
                    for dt in range(NT_C):
                        ps = qmmps.tile([P, CH], F32, tag="projps", name="projps")
                        for ct in range(NT_C):
                            nc.tensor.matmul(
                                ps,
                                WqT[:, ct, dt * P : (dt + 1) * P],
                                xT[:, ct, :],
                                start=(ct == 0),
                                stop=(ct == NT_C - 1),
                            )
                        qt = qst.tile([P, CH], F32R, tag="qt", name="qt")
                        nc.vector.tensor_scalar_add(
                            out=qt, in0=ps, scalar1=bq_sb[:, dt : dt + 1]
                        )
                        nc.scalar.dma_start(out=qT_d[dt, :, s0 : s0 + CH], in_=qt)

        # ---------------- Phase K: project kT (fp32r) -> resident SBUF -------
        res_k = es.enter_context(tc.tile_pool(name="resk", bufs=1))
        kT = res_k.tile([P, NT_C, S], F32R, tag="kT", name="kT")
        with tc.tile_pool(name="wk", bufs=1) as wk_pool:
            WkT = prep_w(Wk, wk_pool, f32r=True)
            with (
                tc.tile_pool(name="kxs", bufs=2) as kxs,
                tc.tile_pool(name="kxt", bufs=2) as kxt,
                tc.tile_pool(name="kmmps", bufs=2, space="PSUM") as kmmps,
            ):
                nxt = prep_xT_f32r(x2, 0, kxs, kxt)
                for ich in range(NCH_S):
                    s0 = ich * CH
                    xT = nxt
                    if ich + 1 < NCH_S:
                        nxt = prep_xT_f32r(x2, (ich + 1) * CH, kxs, kxt)
                    for dt in range(NT_C):
                        ps = kmmps.tile([P, CH], F32, tag="projps", name="kprojps")
                        for ct in range(NT_C):
                            nc.tensor.matmul(
                                ps,
                                WkT[:, ct, dt * P : (dt + 1) * P],
                                xT[:, ct, :],
                                start=(ct == 0),
                                stop=(ct == NT_C - 1),
                            )
                        nc.vector.tensor_scalar_add(
                            out=kT[:, dt, s0 : s0 + CH],
                            in0=ps,
                            scalar1=bk_sb[:, dt : dt + 1],
                        )

        # ---------------- Phase V: project v (natural [s, c], fp16) ----------
        res_v = es.enter_context(tc.tile_pool(name="resv", bufs=1))
        v_r = res_v.tile([P, NT_S, C], F16, tag="v", name="v")
        with tc.tile_pool(name="wv", bufs=1) as wv_pool:
            WvT = prep_w(Wv, wv_pool, f32r=False)
            with (
                tc.tile_pool(name="vxs", bufs=2) as vxs,
                tc.tile_pool(name="vxt", bufs=2) as vxt,
                tc.tile_pool(name="vmmps", bufs=2, space="PSUM") as vmmps,
            ):
                nxt3 = prep_xT_f16(x3, 0, vxs, vxt)
                for ich in range(NCH_S):
                    x3T = nxt3
                    if ich + 1 < NCH_S:
                        nxt3 = prep_xT_f16(x3, (ich + 1) * CH, vxs, vxt)
                    for j in range(CH // P):  # s-tile within chunk
                        st = ich * (CH // P) + j
                        for cch in range(NCH_C):
                            ps = vmmps.tile([P, CH], F32, tag="vps", name="vps")
                            for ct in range(NT_C):
                                nc.tensor.matmul(
                                    ps,
                                    x3T[:, ct, j * P : (j + 1) * P],
                                    WvT[:, ct, cch * CH : (cch + 1) * CH],
                                    start=(ct == 0),
                                    stop=(ct == NT_C - 1),
                                )
                            nc.vector.tensor_tensor(
                                out=v_r[:, st, cch * CH : (cch + 1) * CH],
                                in0=ps,
                                in1=bv_sb[:, cch * CH : (cch + 1) * CH],
                                op=ADD,
                            )

        # ---------------- Attention ------------------------------------------
        with (
            tc.tile_pool(name="qstream", bufs=2) as qstream,
            tc.tile_pool(name="spsum", bufs=6, space="PSUM") as spsum,
            tc.tile_pool(name="opsum", bufs=2, space="PSUM") as opsum,
            tc.tile_pool(name="attn", bufs=2) as attn,
            tc.tile_pool(name="stats", bufs=4) as stats,
        ):
            for sq in range(NT_S):
                qTs = qstream.tile([P, NT_C, P], F32R, tag="qs", name="qs")
                nc.scalar.dma_start(
                    out=qTs,
                    in_=qT_d[:, :, sq * P : (sq + 1) * P].rearrange("t p s -> p t s"),
                )

                # scores: s[sq-tile, :] accumulated over d in 4 chunk banks
                ps_s = [
                    spsum.tile([P, CH], F32, tag="s", name=f"s{c}")
                    for c in range(NCH_S)
                ]
                for dt in range(NT_C):
                    q_sl = qTs[:, dt, :]
                    for c in range(NCH_S):
                        nc.tensor.matmul(
                            ps_s[c],
                            q_sl,
                            kT[:, dt, c * CH : (c + 1) * CH],
                            start=(dt == 0),
                            stop=(dt == NT_C - 1),
                        )

                # softmax (fp32, row-wise over the free dim)
                mx = stats.tile([P, NCH_S], F32, tag="mx", name="mx")
                for c in range(NCH_S):
                    nc.vector.reduce_max(out=mx[:, c : c + 1], in_=ps_s[c], axis=AX)
                negmax = stats.tile([P, 1], F32, tag="negmax", name="negmax")
                nc.vector.reduce_max(out=negmax, in_=mx, axis=AX, negate=True)

                p_sb = attn.tile([P, S], F16, tag="p", name="p")
                sums = stats.tile([P, NCH_S], F32, tag="sums", name="sums")
                for c in range(NCH_S):
                    nc.scalar.activation(
                        out=p_sb[:, c * CH : (c + 1) * CH],
                        in_=ps_s[c],
                        func=EXP,
                        bias=negmax,
                        scale=1.0,
                        accum_out=sums[:, c : c + 1],
                    )
                rs = stats.tile([P, 1], F32, tag="rs", name="rs")
                nc.vector.reduce_sum(out=rs, in_=sums, axis=AX)
                rinv = stats.tile([P, 1], F32, tag="rinv", name="rinv")
                nc.vector.reciprocal(out=rinv, in_=rs)

                # transpose p for attn @ v: one xbar DMA per sq-tile
                pT = attn.tile([P, NT_S, P], F16, tag="pT", name="pT")
                nc.sync.dma_start(out=pT, in_=p_sb, transpose=True)

                # attn @ v, accumulate over sk tiles; normalize; store
                ps_o = [
                    opsum.tile([P, CH], F32, tag="o", name=f"o{c}")
                    for c in range(NCH_C)
                ]
                for skt in range(NT_S):
                    for cch in range(NCH_C):
                        nc.tensor.matmul(
                            ps_o[cch],
                            pT[:, skt, :],
                            v_r[:, skt, cch * CH : (cch + 1) * CH],
                            start=(skt == 0),
                            stop=(skt == NT_S - 1),
                        )
                o_sb = attn.tile([P, C], F32, tag="osb", name="osb")
                for cch in range(NCH_C):
                    nc.vector.tensor_scalar_mul(
                        out=o_sb[:, cch * CH : (cch + 1) * CH],
                        in0=ps_o[cch],
                        scalar1=rinv,
                    )
                nc.scalar.dma_start(out=out[sq * P : (sq + 1) * P, :], in_=o_sb)


_BUILT = {}


def _build():
    if "nc" not in _BUILT:
        nc = bacc.Bacc(
            "TRN2",
            target_bir_lowering=False,
            debug=False,
            num_devices=B,
        )
        with tile.TileContext(nc) as tc:
            _emit(tc)
        nc.compile()
        _BUILT["nc"] = nc
    return _BUILT["nc"]


def kernel_with_results(trace=False, **inputs):
    nc = _build()
    in_maps = []
    for i in range(B):
        in_maps.append(
            {
                "x1": np.ascontiguousarray(inputs["x1"][i], dtype=np.float32),
                "x2": np.ascontiguousarray(inputs["x2"][i], dtype=np.float32),
                "x3": np.ascontiguousarray(inputs["x3"][i], dtype=np.float32),
                "Wq": np.ascontiguousarray(inputs["Wq"], dtype=np.float32),
                "Wk": np.ascontiguousarray(inputs["Wk"], dtype=np.float32),
                "Wv": np.ascontiguousarray(inputs["Wv"], dtype=np.float32),
                "bq": np.ascontiguousarray(inputs["bq"], dtype=np.float32),
                "bk": np.ascontiguousarray(inputs["bk"], dtype=np.float32),
                "bv": np.ascontiguousarray(inputs["bv"], dtype=np.float32),
            }
        )
    res = run_bass_kernel_spmd(nc, in_maps, core_ids=list(range(B)), trace=trace)
    outs = np.stack([r["out"] for r in res.results], axis=0).astype(np.float32)
    return outs, res


def kernel(**inputs):
    outs, _ = kernel_with_results(trace=False, **inputs)
    return outs
